# revision 5
# baseline (speedup 1.0000x reference)
"""Trainium2 Bass kernel for BilinearDecoder.

score = sigmoid( einsum('ed,ed->e', z[edges[0]] @ W, z[edges[1]]) )

Strategy (per core, edges sharded 8 ways):
  Phase 1: zW = z @ W computed once per core (10000x512 @ 512x512),
           written to internal DRAM.  Host passes z pre-transposed
           (zt = z.T) so lhsT tiles load directly, no PE transposes.
  Phase 2: bulk dma_gather of zW[row] and z[col] rows (2KB each) into
           SBUF, fused DVE tensor_tensor_reduce per 128-edge block for
           the per-edge dot product, one sigmoid on ACT, one DMA out.
"""

import sys

if "/opt/trn_rl_repo" not in sys.path:
    sys.path.insert(0, "/opt/trn_rl_repo")

import numpy as np

N_NODES = 10000
N_NODES_PAD = 10240  # pad to multiple of 128
W_DIM = 512
N_EDGES = 131072
N_CORES = 8
EC = N_EDGES // N_CORES  # 16384 edges per core
CHUNK = 1024  # edges per dma_gather (2048 crashes HW: SWDGE ring limit)
NCHUNK = EC // CHUNK  # 8
NBLK = EC // 128  # 128 score columns per core

_cache = {}


def _build():
    import concourse.bacc as bacc
    import concourse.tile as tile
    from concourse import mybir

    f32 = mybir.dt.float32
    i16 = mybir.dt.int16

    nc = bacc.Bacc(
        "TRN2", target_bir_lowering=False, debug=False, num_devices=N_CORES
    )
    zt = nc.dram_tensor("zt", [W_DIM, N_NODES_PAD], f32, kind="ExternalInput")
    ztbl = nc.dram_tensor("ztbl", [N_NODES_PAD, W_DIM], f32, kind="ExternalInput")
    w = nc.dram_tensor("w", [W_DIM, W_DIM], f32, kind="ExternalInput")
    ridx = nc.dram_tensor("ridx", [128, EC // 16], i16, kind="ExternalInput")
    cidx = nc.dram_tensor("cidx", [128, EC // 16], i16, kind="ExternalInput")
    zw = nc.dram_tensor("zw", [N_NODES_PAD, W_DIM], f32, kind="Internal")
    out = nc.dram_tensor("scores", [128, NBLK], f32, kind="ExternalOutput")

    with tile.TileContext(nc) as tc:
        with (
            tc.tile_pool(name="wpool", bufs=1) as wpool,
            tc.tile_pool(name="zpanel", bufs=2) as zpool,
            tc.tile_pool(name="zwstage", bufs=4) as zwpool,
            tc.tile_pool(name="psum", bufs=4, space="PSUM") as psum_pool,
            tc.tile_pool(name="idx", bufs=1) as idxpool,
            tc.tile_pool(name="rgath", bufs=2) as rpool,
            tc.tile_pool(name="cgath", bufs=2) as cpool,
            tc.tile_pool(name="scr", bufs=2) as spool,
            tc.tile_pool(name="misc", bufs=1) as mpool,
        ):
            # W: 4 K-chunk tiles [128, 512]
            w_tiles = []
            for k in range(4):
                wt = wpool.tile([128, W_DIM], f32, tag=f"w{k}")
                nc.sync.dma_start(wt[:], w[k * 128 : (k + 1) * 128, :])
                w_tiles.append(wt)

            # gather index tables
            ridx_sb = idxpool.tile([128, EC // 16], i16, tag="ridx")
            nc.sync.dma_start(ridx_sb[:], ridx[:])
            cidx_sb = idxpool.tile([128, EC // 16], i16, tag="cidx")
            nc.sync.dma_start(cidx_sb[:], cidx[:])

            scores = mpool.tile([128, NBLK], f32, tag="scores")

            # ---- Phase 1: zW = z @ W ----
            PANEL = 512  # nodes per zT panel
            for p in range(N_NODES_PAD // PANEL):
                zp = []
                for k in range(4):
                    t = zpool.tile([128, PANEL], f32, tag=f"zp{k}")
                    nc.sync.dma_start(
                        t[:], zt[k * 128 : (k + 1) * 128, p * PANEL : (p + 1) * PANEL]
                    )
                    zp.append(t)
                for ntile in range(PANEL // 128):
                    ps = psum_pool.tile([128, W_DIM], f32, tag="ps")
                    for k in range(4):
                        nc.tensor.matmul(
                            ps[:],
                            lhsT=zp[k][:, ntile * 128 : (ntile + 1) * 128],
                            rhs=w_tiles[k][:],
                            start=(k == 0),
                            stop=(k == 3),
                        )
                    st = zwpool.tile([128, W_DIM], f32, tag="zwst")
                    nc.vector.tensor_copy(st[:], ps[:])
                    node0 = p * PANEL + ntile * 128
                    nc.sync.dma_start(zw[node0 : node0 + 128, :], st[:])

            # ---- Phase 2: gather + per-edge dot ----
            for ch in range(NCHUNK):
                icol = slice(ch * (CHUNK // 16), (ch + 1) * (CHUNK // 16))
                rt = rpool.tile([128, CHUNK // 128, W_DIM], f32, tag="rowbuf")
                nc.gpsimd.dma_gather(
                    rt[:], zw[:], ridx_sb[:, icol], CHUNK, CHUNK, W_DIM
                )
                ct = cpool.tile([128, CHUNK // 128, W_DIM], f32, tag="colbuf")
                nc.gpsimd.dma_gather(
                    ct[:], ztbl[:], cidx_sb[:, icol], CHUNK, CHUNK, W_DIM
                )
                for b in range(CHUNK // 128):
                    col = ch * (CHUNK // 128) + b
                    # tensor_tensor_reduce crashes HW here; use mul+reduce
                    scr = spool.tile([128, W_DIM], f32, tag="ttr")
                    nc.vector.tensor_mul(scr[:], rt[:, b, :], ct[:, b, :])
                    nc.vector.tensor_reduce(
                        scores[:, col : col + 1],
                        scr[:],
                        axis=mybir.AxisListType.X,
                        op=mybir.AluOpType.add,
                    )

            sig = mpool.tile([128, NBLK], f32, tag="sig")
            nc.scalar.activation(
                sig[:], scores[:], mybir.ActivationFunctionType.Sigmoid
            )
            nc.sync.dma_start(out[:], sig[:])

    nc.compile()
    return nc


def _get_nc():
    if "nc" not in _cache:
        _cache["nc"] = _build()
    return _cache["nc"]


def _wrap_idx(idx):
    """int16 indices -> [128, n/16] layout: index i at [i%16, i//16],
    replicated across the 8 GPSIMD core groups (16 partitions each)."""
    blk = idx.reshape(-1, 16).T.astype(np.int16)  # [16, n/16]
    return np.ascontiguousarray(np.tile(blk, (8, 1)))  # [128, n/16]


def kernel(z, batch_edges, W, _profile=False):
    from concourse.bass_utils import run_bass_kernel_spmd

    z = np.asarray(z, dtype=np.float32)
    W = np.asarray(W, dtype=np.float32)
    be = np.asarray(batch_edges)

    z_pad = np.zeros((N_NODES_PAD, W_DIM), dtype=np.float32)
    z_pad[:N_NODES] = z
    zt_np = np.ascontiguousarray(z_pad.T)

    rows = be[0].astype(np.int16)
    cols = be[1].astype(np.int16)

    in_maps = []
    for c in range(N_CORES):
        sl = slice(c * EC, (c + 1) * EC)
        in_maps.append(
            {
                "zt": zt_np,
                "ztbl": z_pad,
                "w": W,
                "ridx": _wrap_idx(rows[sl]),
                "cidx": _wrap_idx(cols[sl]),
            }
        )

    nc = _get_nc()
    kwargs = {}
    if _profile:
        kwargs = {"trace": True}
    res = run_bass_kernel_spmd(nc, in_maps, core_ids=list(range(N_CORES)), **kwargs)
    _cache["last_res"] = res

    outs = []
    for c in range(N_CORES):
        s = res.results[c]["scores"]  # [128, NBLK]; edge i at [i%128, i//128]
        outs.append(np.ascontiguousarray(s.T).reshape(-1))
    return np.concatenate(outs)


# revision 18
# speedup vs baseline: 1.2690x; 1.2690x over previous
"""Trainium2 Bass kernel for BilinearDecoder.

score = sigmoid( einsum('ed,ed->e', z[edges[0]] @ W, z[edges[1]]) )

Strategy (per core, edges sharded 8 ways):
  Phase 1: zW = z @ W computed once per core (10000x512 @ 512x512),
           written to internal DRAM.  Host passes z pre-transposed
           (zt = z.T) so lhsT tiles load directly, no PE transposes.
  Phase 2: bulk dma_gather of zW[row] and z[col] rows (2KB each) into
           SBUF, fused DVE tensor_tensor_reduce per 128-edge block for
           the per-edge dot product, one sigmoid on ACT, one DMA out.
"""

import sys

if "/opt/trn_rl_repo" not in sys.path:
    sys.path.insert(0, "/opt/trn_rl_repo")

import numpy as np

N_NODES = 10000
N_NODES_PAD = 10240  # pad to multiple of 128
W_DIM = 512
N_EDGES = 131072
N_CORES = 8
EC = N_EDGES // N_CORES  # 16384 edges per core
CHUNK = 1024  # edges per dma_gather (2048 crashes HW: SWDGE ring limit)
NCHUNK = EC // CHUNK  # 8
NBLK = EC // 128  # 128 score columns per core

_cache = {}


def _build():
    import concourse.bacc as bacc
    import concourse.tile as tile
    from concourse import mybir

    f32 = mybir.dt.float32
    f32r = mybir.dt.float32r
    i16 = mybir.dt.int16

    nc = bacc.Bacc(
        "TRN2",
        target_bir_lowering=False,
        debug=False,
        num_devices=N_CORES,
    )
    # zt/w hold e8m11-pre-rounded data (host-side) declared float32r so the
    # PE runs the full-rate fp32r path (1 cyc/row vs fp32's 4).
    zt = nc.dram_tensor("zt", [W_DIM, N_NODES_PAD], f32r, kind="ExternalInput")
    ztbl = nc.dram_tensor("ztbl", [N_NODES_PAD, W_DIM], f32, kind="ExternalInput")
    w = nc.dram_tensor("w", [W_DIM, W_DIM], f32r, kind="ExternalInput")
    ridx = nc.dram_tensor("ridx", [128, EC // 16], i16, kind="ExternalInput")
    cidx = nc.dram_tensor("cidx", [128, EC // 16], i16, kind="ExternalInput")
    zw = nc.dram_tensor("zw", [N_NODES_PAD, W_DIM], f32, kind="Internal")
    out = nc.dram_tensor("scores", [128, NBLK], f32, kind="ExternalOutput")

    with tile.TileContext(nc) as tc:
        with (
            tc.tile_pool(name="wpool", bufs=1) as wpool,
            tc.tile_pool(name="zpanel", bufs=2) as zpool,
            tc.tile_pool(name="zwstage", bufs=4) as zwpool,
            tc.tile_pool(name="psum", bufs=4, space="PSUM") as psum_pool,
            tc.tile_pool(name="idx", bufs=1) as idxpool,
            tc.tile_pool(name="rgath", bufs=3) as rpool,
            tc.tile_pool(name="cgath", bufs=3) as cpool,
            tc.tile_pool(name="scr", bufs=3) as spool,
            tc.tile_pool(name="misc", bufs=1) as mpool,
        ):
            # W: 4 K-chunk tiles [128, 512]
            w_tiles = []
            for k in range(4):
                wt = wpool.tile([128, W_DIM], f32r, tag=f"w{k}")
                nc.sync.dma_start(wt[:], w[k * 128 : (k + 1) * 128, :])
                w_tiles.append(wt)

            # gather index tables
            ridx_sb = idxpool.tile([128, EC // 16], i16, tag="ridx")
            nc.sync.dma_start(ridx_sb[:], ridx[:])
            cidx_sb = idxpool.tile([128, EC // 16], i16, tag="cidx")
            nc.sync.dma_start(cidx_sb[:], cidx[:])

            scores = mpool.tile([128, NBLK], f32, tag="scores")

            # ---- Phase 1: zW = z @ W ----
            PANEL = 512  # nodes per zT panel
            for p in range(N_NODES_PAD // PANEL):
                zp = []
                for k in range(4):
                    t = zpool.tile([128, PANEL], f32r, tag=f"zp{k}")
                    nc.sync.dma_start(
                        t[:], zt[k * 128 : (k + 1) * 128, p * PANEL : (p + 1) * PANEL]
                    )
                    zp.append(t)
                for ntile in range(PANEL // 128):
                    ps = psum_pool.tile([128, W_DIM], f32, tag="ps")
                    for k in range(4):
                        nc.tensor.matmul(
                            ps[:],
                            lhsT=zp[k][:, ntile * 128 : (ntile + 1) * 128],
                            rhs=w_tiles[k][:],
                            start=(k == 0),
                            stop=(k == 3),
                        )
                    st = zwpool.tile([128, W_DIM], f32, tag="zwst")
                    nc.vector.tensor_copy(st[:], ps[:])
                    node0 = p * PANEL + ntile * 128
                    nc.sync.dma_start(zw[node0 : node0 + 128, :], st[:])

            # ---- Phase 2: gather + per-edge dot ----
            for ch in range(NCHUNK):
                icol = slice(ch * (CHUNK // 16), (ch + 1) * (CHUNK // 16))
                rt = rpool.tile([128, CHUNK // 128, W_DIM], f32, tag="rowbuf")
                nc.gpsimd.dma_gather(
                    rt[:], zw[:], ridx_sb[:, icol], CHUNK, CHUNK, W_DIM
                )
                ct = cpool.tile([128, CHUNK // 128, W_DIM], f32, tag="colbuf")
                nc.gpsimd.dma_gather(
                    ct[:], ztbl[:], cidx_sb[:, icol], CHUNK, CHUNK, W_DIM
                )
                for b in range(CHUNK // 128):
                    col = ch * (CHUNK // 128) + b
                    # tensor_tensor_reduce crashes HW here; DVE does the
                    # elementwise mul, ACT does the free-dim reduce via the
                    # activation accumulator (keeps DVE off the critical path)
                    scr = spool.tile([128, W_DIM], f32, tag="ttr")
                    nc.vector.tensor_mul(scr[:], rt[:, b, :], ct[:, b, :])
                    dump = spool.tile([128, W_DIM], f32, tag="dump")
                    nc.scalar.activation(
                        dump[:],
                        scr[:],
                        mybir.ActivationFunctionType.Copy,
                        accum_out=scores[:, col : col + 1],
                    )

            sig = mpool.tile([128, NBLK], f32, tag="sig")
            nc.scalar.activation(
                sig[:], scores[:], mybir.ActivationFunctionType.Sigmoid
            )
            nc.sync.dma_start(out[:], sig[:])

    nc.compile()
    return nc


def _get_nc():
    if "nc" not in _cache:
        _cache["nc"] = _build()
    return _cache["nc"]


def _round_e8m11(x):
    """Round fp32 mantissa to 11 bits (round-half-even at bit 12) — the
    float32r encoding the PE consumes at full rate."""
    b = np.ascontiguousarray(x).view(np.uint32)
    low = b & np.uint32(0xFFF)
    round_up = (low > 0x800) | ((low == 0x800) & ((b >> 12) & 1).astype(bool))
    b = (b & np.uint32(0xFFFFF000)) + (round_up.astype(np.uint32) << 12)
    return b.view(np.float32)


def _wrap_idx(idx):
    """int16 indices -> [128, n/16] layout: index i at [i%16, i//16],
    replicated across the 8 GPSIMD core groups (16 partitions each)."""
    blk = idx.reshape(-1, 16).T.astype(np.int16)  # [16, n/16]
    return np.ascontiguousarray(np.tile(blk, (8, 1)))  # [128, n/16]


def kernel(z, batch_edges, W, _profile=False):
    from concourse.bass_utils import run_bass_kernel_spmd

    z = np.asarray(z, dtype=np.float32)
    W = np.asarray(W, dtype=np.float32)
    be = np.asarray(batch_edges)

    z_pad = np.zeros((N_NODES_PAD, W_DIM), dtype=np.float32)
    z_pad[:N_NODES] = z
    zt_np = _round_e8m11(np.ascontiguousarray(z_pad.T))
    w_r = _round_e8m11(W)

    rows = be[0].astype(np.int16)
    cols = be[1].astype(np.int16)

    in_maps = []
    for c in range(N_CORES):
        sl = slice(c * EC, (c + 1) * EC)
        in_maps.append(
            {
                "zt": zt_np,
                "ztbl": z_pad,
                "w": w_r,
                "ridx": _wrap_idx(rows[sl]),
                "cidx": _wrap_idx(cols[sl]),
            }
        )

    nc = _get_nc()
    kwargs = {}
    if _profile:
        kwargs = {"trace": True}
    res = run_bass_kernel_spmd(nc, in_maps, core_ids=list(range(N_CORES)), **kwargs)
    _cache["last_res"] = res

    outs = []
    for c in range(N_CORES):
        s = res.results[c]["scores"]  # [128, NBLK]; edge i at [i%128, i//128]
        outs.append(np.ascontiguousarray(s.T).reshape(-1))
    return np.concatenate(outs)


# revision 22
# speedup vs baseline: 1.4715x; 1.1596x over previous
"""Trainium2 Bass kernel for BilinearDecoder.

score = sigmoid( einsum('ed,ed->e', z[edges[0]] @ W, z[edges[1]]) )

Strategy (per core, edges sharded 8 ways):
  Phase 1: zW = z @ W computed once per core (10000x512 @ 512x512),
           written to internal DRAM.  Host passes z pre-transposed
           (zt = z.T) so lhsT tiles load directly, no PE transposes.
  Phase 2: bulk dma_gather of zW[row] and z[col] rows (2KB each) into
           SBUF, fused DVE tensor_tensor_reduce per 128-edge block for
           the per-edge dot product, one sigmoid on ACT, one DMA out.
"""

import sys

if "/opt/trn_rl_repo" not in sys.path:
    sys.path.insert(0, "/opt/trn_rl_repo")

import numpy as np

N_NODES = 10000
N_NODES_PAD = 10240  # pad to multiple of 128
W_DIM = 512
N_EDGES = 131072
N_CORES = 8
EC = N_EDGES // N_CORES  # 16384 edges per core
CHUNK = 1024  # edges per dma_gather (2048 crashes HW: SWDGE ring limit)
NCHUNK = EC // CHUNK  # 8
NBLK = EC // 128  # 128 score columns per core

_cache = {}


def _chunk_bounds():
    """Static per-chunk zw prefix bounds (in nodes, multiple of 128).

    Host sorts each core's edges by row index, so row-gather chunk k only
    reads zw rows below roughly the (k+1)/NCHUNK quantile.  The +768
    margin is >10 sigma of the order-statistic fluctuation for uniform
    indices; the host verifies per input and falls back to full bounds."""
    bs = []
    for k in range(NCHUNK):
        b = int(np.ceil((N_NODES_PAD * (k + 1) / NCHUNK + 768) / 128.0) * 128)
        bs.append(min(N_NODES_PAD, b))
    return bs


def _build(sorted_rows=True):
    import concourse.bacc as bacc
    import concourse.tile as tile
    from concourse import mybir

    f32 = mybir.dt.float32
    f32r = mybir.dt.float32r
    i16 = mybir.dt.int16

    nc = bacc.Bacc(
        "TRN2",
        target_bir_lowering=False,
        debug=False,
        num_devices=N_CORES,
    )
    # zt/w hold e8m11-pre-rounded data (host-side) declared float32r so the
    # PE runs the full-rate fp32r path (1 cyc/row vs fp32's 4).
    zt = nc.dram_tensor("zt", [W_DIM, N_NODES_PAD], f32r, kind="ExternalInput")
    ztbl = nc.dram_tensor("ztbl", [N_NODES_PAD, W_DIM], f32, kind="ExternalInput")
    w = nc.dram_tensor("w", [W_DIM, W_DIM], f32r, kind="ExternalInput")
    ridx = nc.dram_tensor("ridx", [128, EC // 16], i16, kind="ExternalInput")
    cidx = nc.dram_tensor("cidx", [128, EC // 16], i16, kind="ExternalInput")
    zw = nc.dram_tensor("zw", [N_NODES_PAD, W_DIM], f32, kind="Internal")
    out = nc.dram_tensor("scores", [128, NBLK], f32, kind="ExternalOutput")

    with tile.TileContext(nc) as tc:
        with (
            tc.tile_pool(name="wpool", bufs=1) as wpool,
            tc.tile_pool(name="zpanel", bufs=2) as zpool,
            tc.tile_pool(name="zwstage", bufs=4) as zwpool,
            tc.tile_pool(name="psum", bufs=4, space="PSUM") as psum_pool,
            tc.tile_pool(name="idx", bufs=1) as idxpool,
            tc.tile_pool(name="rgath", bufs=3) as rpool,
            tc.tile_pool(name="cgath", bufs=3) as cpool,
            tc.tile_pool(name="scr", bufs=3) as spool,
            tc.tile_pool(name="misc", bufs=1) as mpool,
        ):
            # W: 4 K-chunk tiles [128, 512]
            w_tiles = []
            for k in range(4):
                wt = wpool.tile([128, W_DIM], f32r, tag=f"w{k}")
                nc.sync.dma_start(wt[:], w[k * 128 : (k + 1) * 128, :])
                w_tiles.append(wt)

            # gather index tables
            ridx_sb = idxpool.tile([128, EC // 16], i16, tag="ridx")
            nc.sync.dma_start(ridx_sb[:], ridx[:])
            cidx_sb = idxpool.tile([128, EC // 16], i16, tag="cidx")
            nc.sync.dma_start(cidx_sb[:], cidx[:])

            scores = mpool.tile([128, NBLK], f32, tag="scores")

            # ---- Phase 1: zW = z @ W ----
            PANEL = 512  # nodes per zT panel
            for p in range(N_NODES_PAD // PANEL):
                zp = []
                for k in range(4):
                    t = zpool.tile([128, PANEL], f32r, tag=f"zp{k}")
                    nc.sync.dma_start(
                        t[:], zt[k * 128 : (k + 1) * 128, p * PANEL : (p + 1) * PANEL]
                    )
                    zp.append(t)
                for ntile in range(PANEL // 128):
                    ps = psum_pool.tile([128, W_DIM], f32, tag="ps")
                    for k in range(4):
                        nc.tensor.matmul(
                            ps[:],
                            lhsT=zp[k][:, ntile * 128 : (ntile + 1) * 128],
                            rhs=w_tiles[k][:],
                            start=(k == 0),
                            stop=(k == 3),
                        )
                    st = zwpool.tile([128, W_DIM], f32, tag="zwst")
                    nc.vector.tensor_copy(st[:], ps[:])
                    node0 = p * PANEL + ntile * 128
                    nc.sync.dma_start(zw[node0 : node0 + 128, :], st[:])

            # ---- Phase 2: gather + per-edge dot ----
            bounds = _chunk_bounds() if sorted_rows else [N_NODES_PAD] * NCHUNK
            for ch in range(NCHUNK):
                icol = slice(ch * (CHUNK // 16), (ch + 1) * (CHUNK // 16))
                rt = rpool.tile([128, CHUNK // 128, W_DIM], f32, tag="rowbuf")
                # Sliced zw source: Tile's range-granular DRAM deps let this
                # gather start as soon as the needed zw prefix is written,
                # overlapping row gathers with the phase-1 matmul.
                nc.gpsimd.dma_gather(
                    rt[:], zw[: bounds[ch], :], ridx_sb[:, icol], CHUNK, CHUNK, W_DIM
                )
                ct = cpool.tile([128, CHUNK // 128, W_DIM], f32, tag="colbuf")
                nc.gpsimd.dma_gather(
                    ct[:], ztbl[:], cidx_sb[:, icol], CHUNK, CHUNK, W_DIM
                )
                for b in range(CHUNK // 128):
                    col = ch * (CHUNK // 128) + b
                    # tensor_tensor_reduce crashes HW here; DVE does the
                    # elementwise mul, ACT does the free-dim reduce via the
                    # activation accumulator (keeps DVE off the critical path)
                    scr = spool.tile([128, W_DIM], f32, tag="ttr")
                    nc.vector.tensor_mul(scr[:], rt[:, b, :], ct[:, b, :])
                    dump = spool.tile([128, W_DIM], f32, tag="dump")
                    nc.scalar.activation(
                        dump[:],
                        scr[:],
                        mybir.ActivationFunctionType.Copy,
                        accum_out=scores[:, col : col + 1],
                    )

            sig = mpool.tile([128, NBLK], f32, tag="sig")
            nc.scalar.activation(
                sig[:], scores[:], mybir.ActivationFunctionType.Sigmoid
            )
            nc.sync.dma_start(out[:], sig[:])

    nc.compile()
    return nc


def _get_nc(sorted_rows=True):
    key = f"nc_{sorted_rows}"
    if key not in _cache:
        _cache[key] = _build(sorted_rows)
    return _cache[key]


def _round_e8m11(x):
    """Round fp32 mantissa to 11 bits (round-half-even at bit 12) — the
    float32r encoding the PE consumes at full rate."""
    b = np.ascontiguousarray(x).view(np.uint32)
    low = b & np.uint32(0xFFF)
    round_up = (low > 0x800) | ((low == 0x800) & ((b >> 12) & 1).astype(bool))
    b = (b & np.uint32(0xFFFFF000)) + (round_up.astype(np.uint32) << 12)
    return b.view(np.float32)


def _wrap_idx(idx):
    """int16 indices -> [128, n/16] layout: index i at [i%16, i//16],
    replicated across the 8 GPSIMD core groups (16 partitions each)."""
    blk = idx.reshape(-1, 16).T.astype(np.int16)  # [16, n/16]
    return np.ascontiguousarray(np.tile(blk, (8, 1)))  # [128, n/16]


def kernel(z, batch_edges, W, _profile=False):
    from concourse.bass_utils import run_bass_kernel_spmd

    z = np.asarray(z, dtype=np.float32)
    W = np.asarray(W, dtype=np.float32)
    be = np.asarray(batch_edges)

    z_pad = np.zeros((N_NODES_PAD, W_DIM), dtype=np.float32)
    z_pad[:N_NODES] = z
    zt_np = _round_e8m11(np.ascontiguousarray(z_pad.T))
    w_r = _round_e8m11(W)

    rows = be[0].astype(np.int16)
    cols = be[1].astype(np.int16)

    bounds = _chunk_bounds()
    in_maps = []
    orders = []
    sorted_ok = True
    for c in range(N_CORES):
        sl = slice(c * EC, (c + 1) * EC)
        order = np.argsort(rows[sl], kind="stable")
        r_s = rows[sl][order]
        c_s = cols[sl][order]
        orders.append(order)
        for k in range(NCHUNK):
            if r_s[k * CHUNK : (k + 1) * CHUNK].max() >= bounds[k]:
                sorted_ok = False
        in_maps.append(
            {
                "zt": zt_np,
                "ztbl": z_pad,
                "w": w_r,
                "ridx": _wrap_idx(r_s),
                "cidx": _wrap_idx(c_s),
            }
        )

    nc = _get_nc(sorted_rows=sorted_ok)
    kwargs = {}
    if _profile:
        kwargs = {"trace": True}
    res = run_bass_kernel_spmd(nc, in_maps, core_ids=list(range(N_CORES)), **kwargs)
    _cache["last_res"] = res

    outs = []
    for c in range(N_CORES):
        s = res.results[c]["scores"]  # [128, NBLK]; sorted edge i at [i%128, i//128]
        flat = np.ascontiguousarray(s.T).reshape(-1)
        unsorted = np.empty_like(flat)
        unsorted[orders[c]] = flat
        outs.append(unsorted)
    return np.concatenate(outs)


# revision 27
# speedup vs baseline: 1.7878x; 1.2150x over previous
"""Trainium2 Bass kernel for BilinearDecoder.

score = sigmoid( einsum('ed,ed->e', z[edges[0]] @ W, z[edges[1]]) )

Strategy (per core, edges sharded 8 ways):
  Phase 1: zW = z @ W computed once per core (10000x512 @ 512x512),
           written to internal DRAM.  Host passes z pre-transposed
           (zt = z.T) so lhsT tiles load directly, no PE transposes.
  Phase 2: bulk dma_gather of zW[row] and z[col] rows (2KB each) into
           SBUF, fused DVE tensor_tensor_reduce per 128-edge block for
           the per-edge dot product, one sigmoid on ACT, one DMA out.
"""

import sys

if "/opt/trn_rl_repo" not in sys.path:
    sys.path.insert(0, "/opt/trn_rl_repo")

import numpy as np

N_NODES = 10000
N_NODES_PAD = 10240  # pad to multiple of 128
W_DIM = 512
N_EDGES = 131072
N_CORES = 8
EC = N_EDGES // N_CORES  # 16384 edges per core
CHUNK = 1024  # edges per dma_gather (2048 crashes HW: SWDGE ring limit)
NCHUNK = EC // CHUNK  # 8
NBLK = EC // 128  # 128 score columns per core

_cache = {}


def _chunk_bounds():
    """Static per-chunk zw prefix bounds (in nodes, multiple of 128).

    Host sorts each core's edges by row index, so row-gather chunk k only
    reads zw rows below roughly the (k+1)/NCHUNK quantile.  The +768
    margin is >10 sigma of the order-statistic fluctuation for uniform
    indices; the host verifies per input and falls back to full bounds."""
    bs = []
    for k in range(NCHUNK):
        b = int(np.ceil((N_NODES_PAD * (k + 1) / NCHUNK + 768) / 128.0) * 128)
        bs.append(min(N_NODES_PAD, b))
    return bs


def _build(sorted_rows=True):
    import concourse.bacc as bacc
    import concourse.tile as tile
    from concourse import mybir

    f32 = mybir.dt.float32
    f32r = mybir.dt.float32r
    f16 = mybir.dt.float16
    i16 = mybir.dt.int16

    nc = bacc.Bacc(
        "TRN2",
        target_bir_lowering=False,
        debug=False,
        num_devices=N_CORES,
    )
    # zt/w hold e8m11-pre-rounded data (host-side) declared float32r so the
    # PE runs the full-rate fp32r path (1 cyc/row vs fp32's 4).
    zt = nc.dram_tensor("zt", [W_DIM, N_NODES_PAD], f32r, kind="ExternalInput")
    # gather tables are f16 to halve gather bytes (the kernel bottleneck)
    ztbl = nc.dram_tensor("ztbl", [N_NODES_PAD, W_DIM], f16, kind="ExternalInput")
    w = nc.dram_tensor("w", [W_DIM, W_DIM], f32r, kind="ExternalInput")
    ridx = nc.dram_tensor("ridx", [128, EC // 16], i16, kind="ExternalInput")
    cidx = nc.dram_tensor("cidx", [128, EC // 16], i16, kind="ExternalInput")
    zw = nc.dram_tensor("zw", [N_NODES_PAD, W_DIM], f16, kind="Internal")
    out = nc.dram_tensor("scores", [128, NBLK], f32, kind="ExternalOutput")

    with tile.TileContext(nc) as tc:
        with (
            tc.tile_pool(name="wpool", bufs=1) as wpool,
            tc.tile_pool(name="zpanel", bufs=2) as zpool,
            tc.tile_pool(name="zwstage", bufs=4) as zwpool,
            tc.tile_pool(name="psum", bufs=4, space="PSUM") as psum_pool,
            tc.tile_pool(name="idx", bufs=1) as idxpool,
            tc.tile_pool(name="rgath", bufs=3) as rpool,
            tc.tile_pool(name="cgath", bufs=3) as cpool,
            tc.tile_pool(name="scr", bufs=3) as spool,
            tc.tile_pool(name="misc", bufs=1) as mpool,
        ):
            # W: 4 K-chunk tiles [128, 512]
            w_tiles = []
            for k in range(4):
                wt = wpool.tile([128, W_DIM], f32r, tag=f"w{k}")
                nc.sync.dma_start(wt[:], w[k * 128 : (k + 1) * 128, :])
                w_tiles.append(wt)

            # gather index tables
            ridx_sb = idxpool.tile([128, EC // 16], i16, tag="ridx")
            nc.sync.dma_start(ridx_sb[:], ridx[:])
            cidx_sb = idxpool.tile([128, EC // 16], i16, tag="cidx")
            nc.sync.dma_start(cidx_sb[:], cidx[:])

            scores = mpool.tile([128, NBLK], f32, tag="scores")

            # ---- Phase 1: zW = z @ W ----
            PANEL = 512  # nodes per zT panel
            for p in range(N_NODES_PAD // PANEL):
                zp = []
                for k in range(4):
                    t = zpool.tile([128, PANEL], f32r, tag=f"zp{k}")
                    nc.sync.dma_start(
                        t[:], zt[k * 128 : (k + 1) * 128, p * PANEL : (p + 1) * PANEL]
                    )
                    zp.append(t)
                for ntile in range(PANEL // 128):
                    ps = psum_pool.tile([128, W_DIM], f32, tag="ps")
                    for k in range(4):
                        nc.tensor.matmul(
                            ps[:],
                            lhsT=zp[k][:, ntile * 128 : (ntile + 1) * 128],
                            rhs=w_tiles[k][:],
                            start=(k == 0),
                            stop=(k == 3),
                        )
                    st = zwpool.tile([128, W_DIM], f16, tag="zwst")
                    nc.vector.tensor_copy(st[:], ps[:])
                    node0 = p * PANEL + ntile * 128
                    nc.sync.dma_start(zw[node0 : node0 + 128, :], st[:])

            # ---- Phase 2: gather + per-edge dot ----
            bounds = _chunk_bounds() if sorted_rows else [N_NODES_PAD] * NCHUNK
            for ch in range(NCHUNK):
                icol = slice(ch * (CHUNK // 16), (ch + 1) * (CHUNK // 16))
                rt = rpool.tile([128, CHUNK // 128, W_DIM], f16, tag="rowbuf")
                # Sliced zw source: Tile's range-granular DRAM deps let this
                # gather start as soon as the needed zw prefix is written,
                # overlapping row gathers with the phase-1 matmul.
                nc.gpsimd.dma_gather(
                    rt[:], zw[: bounds[ch], :], ridx_sb[:, icol], CHUNK, CHUNK, W_DIM
                )
                ct = cpool.tile([128, CHUNK // 128, W_DIM], f16, tag="colbuf")
                nc.gpsimd.dma_gather(
                    ct[:], ztbl[:], cidx_sb[:, icol], CHUNK, CHUNK, W_DIM
                )
                for b in range(CHUNK // 128):
                    col = ch * (CHUNK // 128) + b
                    # tensor_tensor_reduce crashes HW here; DVE does the
                    # elementwise mul, ACT does the free-dim reduce via the
                    # activation accumulator (keeps DVE off the critical path)
                    scr = spool.tile([128, W_DIM], f32, tag="ttr")
                    nc.vector.tensor_mul(scr[:], rt[:, b, :], ct[:, b, :])
                    dump = spool.tile([128, W_DIM], f32, tag="dump")
                    nc.scalar.activation(
                        dump[:],
                        scr[:],
                        mybir.ActivationFunctionType.Copy,
                        accum_out=scores[:, col : col + 1],
                    )

            sig = mpool.tile([128, NBLK], f32, tag="sig")
            nc.scalar.activation(
                sig[:], scores[:], mybir.ActivationFunctionType.Sigmoid
            )
            nc.sync.dma_start(out[:], sig[:])

    nc.compile()
    return nc


def _get_nc(sorted_rows=True):
    key = f"nc_{sorted_rows}"
    if key not in _cache:
        _cache[key] = _build(sorted_rows)
    return _cache[key]


def _round_e8m11(x):
    """Round fp32 mantissa to 11 bits (round-half-even at bit 12) — the
    float32r encoding the PE consumes at full rate."""
    b = np.ascontiguousarray(x).view(np.uint32)
    low = b & np.uint32(0xFFF)
    round_up = (low > 0x800) | ((low == 0x800) & ((b >> 12) & 1).astype(bool))
    b = (b & np.uint32(0xFFFFF000)) + (round_up.astype(np.uint32) << 12)
    return b.view(np.float32)


def _wrap_idx(idx):
    """int16 indices -> [128, n/16] layout: index i at [i%16, i//16],
    replicated across the 8 GPSIMD core groups (16 partitions each)."""
    blk = idx.reshape(-1, 16).T.astype(np.int16)  # [16, n/16]
    return np.ascontiguousarray(np.tile(blk, (8, 1)))  # [128, n/16]


def kernel(z, batch_edges, W, _profile=False):
    from concourse.bass_utils import run_bass_kernel_spmd

    z = np.asarray(z, dtype=np.float32)
    W = np.asarray(W, dtype=np.float32)
    be = np.asarray(batch_edges)

    z_pad = np.zeros((N_NODES_PAD, W_DIM), dtype=np.float32)
    z_pad[:N_NODES] = z
    zt_np = _round_e8m11(np.ascontiguousarray(z_pad.T))
    w_r = _round_e8m11(W)
    ztbl_np = z_pad.astype(np.float16)

    rows = be[0].astype(np.int16)
    cols = be[1].astype(np.int16)

    bounds = _chunk_bounds()
    in_maps = []
    orders = []
    sorted_ok = True
    for c in range(N_CORES):
        sl = slice(c * EC, (c + 1) * EC)
        order = np.argsort(rows[sl], kind="stable")
        r_s = rows[sl][order]
        c_s = cols[sl][order]
        orders.append(order)
        for k in range(NCHUNK):
            if r_s[k * CHUNK : (k + 1) * CHUNK].max() >= bounds[k]:
                sorted_ok = False
        in_maps.append(
            {
                "zt": zt_np,
                "ztbl": ztbl_np,
                "w": w_r,
                "ridx": _wrap_idx(r_s),
                "cidx": _wrap_idx(c_s),
            }
        )

    nc = _get_nc(sorted_rows=sorted_ok)
    kwargs = {}
    if _profile:
        kwargs = {"trace": True}
    res = run_bass_kernel_spmd(nc, in_maps, core_ids=list(range(N_CORES)), **kwargs)
    _cache["last_res"] = res

    outs = []
    for c in range(N_CORES):
        s = res.results[c]["scores"]  # [128, NBLK]; sorted edge i at [i%128, i//128]
        flat = np.ascontiguousarray(s.T).reshape(-1)
        unsorted = np.empty_like(flat)
        unsorted[orders[c]] = flat
        outs.append(unsorted)
    return np.concatenate(outs)
